# revision 13
# baseline (speedup 1.0000x reference)
"""Bass/Tile kernel for sparse sliding-window attention with sinks (v2).

Problem (full): B=4, N=1024, DIM=1024, H=16, D=64, SW=256.
Sharding: 8 cores; core c -> batch b=c//2, head-group g=c%2 (8 heads each).
Host sums the two per-head-group partial projections + proj bias.

v2 changes vs baseline:
  - mean-centering folded into qkv weights on host (LN mean path removed;
    var = sumsq/D on centered z)
  - fp16 for all post-QKV tensors (zraw/qkt/v/p/att/proj weights) -> DVE
    2x/4x modes, PE 1 cyc/row
  - rstd via Quake rsqrt on DVE (no ACT Sqrt -> single act table set);
    q-side rstd folded into the softmax exp scale (free), k-side applied
    once per tile with sqrt(D) folded into the k rope tables
  - sliding-window masks applied via a PE accumulation matmul
    (identity-stationary, host mask tile moving) instead of DVE adds
  - softmax 1/den folded into the p-transpose eviction copy
  - RoPE rotate-half via strided DVE reads (no gpsimd copy)
"""

import sys

sys.path.insert(0, "/opt/trn_rl_repo")

import numpy as np
import ml_dtypes

import concourse.bass as bass
import concourse.mybir as mybir
import concourse.tile as tile
from concourse import bacc

F32 = mybir.dt.float32
F16 = mybir.dt.float16
BF16 = mybir.dt.bfloat16
I32 = mybir.dt.int32

B, N, DIM = 4, 1024, 1024
H, D = 16, 64
SW = 256
ROPE_BASE = 10000.0
LN_EPS = 1e-5
P = 128
NT = N // P      # 8 query/n tiles
CC = DIM // P    # 8 contraction chunks
HL = H // 2      # 8 local heads
NEG = -30000.0   # fp16-safe mask value
MAGIC = 0x5EF759DF  # quake magic adjusted for vh = v/2 input


def build_nc(repeat=1, use_for_i=False, phases="ABC"):
    nc = bacc.Bacc("TRN2", target_bir_lowering=False, debug=False, num_devices=8)

    xt = nc.declare_dram_parameter("xt", [DIM, N], BF16, isOutput=False)
    wqkt = nc.declare_dram_parameter("wqkt", [DIM, 1024], BF16, isOutput=False)
    wvt = nc.declare_dram_parameter("wvt", [DIM, 512], BF16, isOutput=False)
    projt = nc.declare_dram_parameter("projt", [512, DIM], F16, isOutput=False)
    coswq = nc.declare_dram_parameter("coswq", [N, D], F16, isOutput=False)
    sinwq = nc.declare_dram_parameter("sinwq", [N, D], F16, isOutput=False)
    coswk = nc.declare_dram_parameter("coswk", [N, D], F16, isOutput=False)
    sinwk = nc.declare_dram_parameter("sinwk", [N, D], F16, isOutput=False)
    esink = nc.declare_dram_parameter("esink", [1, HL], F32, isOutput=False)
    masks = nc.declare_dram_parameter("masks", [P, 3 * P], F16, isOutput=False)
    identh = nc.declare_dram_parameter("identh", [P, P], F16, isOutput=False)
    y = nc.declare_dram_parameter("y", [N, DIM], F32, isOutput=True)

    with tile.TileContext(nc) as tc:
        with tc.tile_pool(name="consts", bufs=1) as consts:
            wqk_sb = consts.tile([P, CC, 1024], BF16, tag="wqk")
            wqk_src = wqkt.ap().rearrange("(cc p) f -> p cc f", p=P)
            wv_sb = consts.tile([P, CC, 512], BF16, tag="wv")
            wv_src = wvt.ap().rearrange("(cc p) f -> p cc f", p=P)
            for c in range(CC):
                nc.sync.dma_start(out=wqk_sb[:, c, :], in_=wqk_src[:, c, :])
                nc.sync.dma_start(out=wv_sb[:, c, :], in_=wv_src[:, c, :])
            pj_sb = consts.tile([P, 4, DIM], F16, tag="pj")
            pj_src = projt.ap().rearrange("(ch p) e -> p ch e", p=P)
            for ch in range(4):
                nc.sync.dma_start(out=pj_sb[:, ch, :], in_=pj_src[:, ch, :])
            cos_sin = {}
            for nm, t_dram in (
                ("cq", coswq), ("sq", sinwq), ("ck", coswk), ("sk", sinwk),
            ):
                t_sb = consts.tile([P, NT, D], F16, tag=nm)
                nc.sync.dma_start(
                    out=t_sb, in_=t_dram.ap().rearrange("(t p) d -> p t d", p=P))
                cos_sin[nm] = t_sb
            es_sb = consts.tile([P, HL], F32, tag="es")
            nc.sync.dma_start(out=es_sb, in_=esink.ap().to_broadcast([P, HL]))
            mk_sb = consts.tile([P, 3 * P], F16, tag="mk")
            nc.sync.dma_start(out=mk_sb, in_=masks.ap())
            idh_sb = consts.tile([P, P], F16, tag="idh")
            nc.sync.dma_start(out=idh_sb, in_=identh.ap())
            magic_sb = consts.tile([P, 1], I32, tag="magic")
            nc.vector.memset(magic_sb, MAGIC)

            # persistent intermediates
            qkt_sb = consts.tile([P, 8, N], F16, tag="qkt")   # [d, pair, n]
            v_sb = consts.tile([P, NT, 512], F16, tag="v")    # [n, ktile, hd]
            att_sb = consts.tile([P, 4, NT, P], F16, tag="att")  # [hd, pair, qi, n]
            zraw = consts.tile([P, NT, 1024], F16, tag="zraw")   # centered qk
            sq_g = consts.tile([P, NT, 16], F32, tag="sqg")      # sumsq
            y_g = consts.tile([P, NT, 16], F32, tag="yg")        # rsqrt(sumsq+Deps)

            def body(phases=phases):
                with (
                    tc.tile_pool(name="pA", bufs=3) as pA,
                    tc.tile_pool(name="psA", bufs=2, space="PSUM") as psA,
                    tc.tile_pool(name="pR", bufs=1) as pR,
                ):
                    do_ln = ('L' in phases) or ('A' in phases)

                    def a1_tile(t):
                        nsl = slice(t * P, (t + 1) * P)
                        xt_t = pA.tile([P, CC, P], BF16, tag="xt", name=f"xt{t}")
                        nc.sync.dma_start(
                            out=xt_t,
                            in_=xt[:, nsl].rearrange("(cc p) n -> p cc n", p=P))
                        ps_q = psA.tile([P, 512], F32, tag="psq", name=f"psq{t}")
                        ps_k = psA.tile([P, 512], F32, tag="psk", name=f"psk{t}")
                        ps_v = psA.tile([P, 512], F32, tag="psv", name=f"psv{t}")
                        for c in range(CC):
                            st, sp = (c == 0), (c == CC - 1)
                            nc.tensor.matmul(ps_q, xt_t[:, c, :], wqk_sb[:, c, 0:512],
                                             start=st, stop=sp)
                            nc.tensor.matmul(ps_k, xt_t[:, c, :], wqk_sb[:, c, 512:1024],
                                             start=st, stop=sp)
                            nc.tensor.matmul(ps_v, xt_t[:, c, :], wv_sb[:, c, :],
                                             start=st, stop=sp)
                        nc.scalar.copy(out=v_sb[:, t, :], in_=ps_v)
                        if not do_ln:
                            return
                        # evict centered z to fp16 (ACT), sumsq via DVE on fp16
                        nc.scalar.copy(out=zraw[:, t, 0:512], in_=ps_q)
                        nc.scalar.copy(out=zraw[:, t, 512:1024], in_=ps_k)
                        sq = pR.tile([P, 16, D], F16, tag=f"sq{t % 2}",
                                     name=f"sq{t}")
                        z16 = zraw[:, t, :].rearrange("p (s d) -> p s d", d=D)
                        nc.vector.tensor_mul(out=sq, in0=z16, in1=z16)
                        with nc.allow_low_precision("fp16 sumsq ok for rstd"):
                            nc.vector.tensor_reduce(
                                out=sq_g[:, t, :], in_=sq,
                                axis=mybir.AxisListType.X, op=mybir.AluOpType.add)

                    def stats_batch(g):
                        # y = rsqrt(sumsq + D*eps) for tiles [4g, 4g+4) via
                        # quake rsqrt + 2 newton steps (all DVE, no ACT table)
                        tsl = slice(g * 4, g * 4 + 4)
                        ysl = y_g[:, tsl, :]
                        vh = pR.tile([P, 4, 16], F32, tag=f"vh{g}", name=f"vh{g}")
                        yt = pR.tile([P, 4, 16], F32, tag=f"yt{g}", name=f"yt{g}")
                        sc2 = pR.tile([P, 4, 16], F32, tag=f"sc2{g}", name=f"sc2{g}")
                        nc.vector.tensor_scalar(
                            out=vh, in0=sq_g[:, tsl, :], scalar1=0.5,
                            scalar2=0.5 * D * LN_EPS,
                            op0=mybir.AluOpType.mult, op1=mybir.AluOpType.add)
                        ivh = vh.bitcast(I32)
                        iy = yt.bitcast(I32)
                        nc.vector.tensor_scalar(
                            out=iy, in0=ivh, scalar1=1, scalar2=None,
                            op0=mybir.AluOpType.logical_shift_right)
                        nc.vector.tensor_tensor(
                            out=iy, in0=magic_sb.broadcast_to([P, 4, 16]),
                            in1=iy, op=mybir.AluOpType.subtract)
                        for it in range(2):
                            dst = ysl if it == 1 else yt
                            nc.vector.tensor_mul(out=sc2, in0=yt, in1=yt)
                            nc.vector.tensor_mul(out=sc2, in0=sc2, in1=vh)
                            nc.vector.tensor_scalar(
                                out=sc2, in0=sc2, scalar1=-1.0, scalar2=1.5,
                                op0=mybir.AluOpType.mult, op1=mybir.AluOpType.add)
                            nc.vector.tensor_mul(out=dst, in0=yt, in1=sc2)

                    def a3_tiles(ts):
                        for t in ts:
                            rot = pR.tile([P, 1024], F16, tag=f"rot{t % 2}",
                                          name=f"rotb{t}")
                            tmp = pR.tile([P, 1024], F16, tag=f"tmp{t % 2}",
                                          name=f"tmpb{t}")
                            J = D // 2
                            zv = zraw[:, t, :].rearrange(
                                "p (s h j) -> p s h j", h=2, j=J)
                            rv = rot.rearrange("p (s h j) -> p s h j", h=2, j=J)
                            tv = tmp.rearrange("p (s d) -> p s d", d=D)
                            for half, (cn, sn) in enumerate(
                                    (("cq", "sq"), ("ck", "sk"))):
                                ssl = slice(half * 8, half * 8 + 8)
                                cb = cos_sin[cn][:, t, :].unsqueeze(1)\
                                    .broadcast_to([P, 8, D])
                                sb3 = cos_sin[sn][:, t, :]\
                                    .rearrange("p (h j) -> p h j", j=J)
                                # t = z * cos
                                nc.vector.tensor_mul(
                                    out=tv[:, ssl, :],
                                    in0=zraw[:, t, half * 512:(half + 1) * 512]
                                    .rearrange("p (s d) -> p s d", d=D),
                                    in1=cb)
                                # r[j'] = z[1-j'] * sin[j']  (rotate-half)
                                for jp in range(2):
                                    nc.vector.tensor_mul(
                                        out=rv[:, ssl, jp, :],
                                        in0=zv[:, ssl, 1 - jp, :],
                                        in1=sb3[:, jp, :].unsqueeze(1)
                                        .broadcast_to([P, 8, J]))
                            # u = t + r (gpsimd), k-half scaled by y_k after
                            nc.gpsimd.tensor_tensor(
                                out=zraw[:, t, :], in0=tmp, in1=rot,
                                op=mybir.AluOpType.add)
                            # k *= y_k (8/sqrt fold in tables)
                            zk = zraw[:, t, 512:1024].rearrange(
                                "p (s d) -> p s d", d=D)
                            nc.vector.tensor_mul(
                                out=zk, in0=zk,
                                in1=y_g[:, t, 8:16].unsqueeze(2)
                                .broadcast_to([P, 8, D]))
                        for t in ts:
                            nsl = slice(t * P, (t + 1) * P)
                            for hp in range(2):  # q half / k half
                                tp = psA.tile([P, 512], F16, tag="tp",
                                              name=f"tp{t}_{hp}")
                                for pr in range(4):
                                    blk = hp * 4 + pr
                                    nc.tensor.transpose(
                                        tp[:, pr * P:(pr + 1) * P],
                                        zraw[:, t, blk * P:(blk + 1) * P],
                                        idh_sb)
                                # evict 4 transposed blocks to qkt
                                for pr in range(4):
                                    if hp == 0:
                                        nc.scalar.copy(
                                            out=qkt_sb[:, hp * 4 + pr, nsl],
                                            in_=tp[:, pr * P:(pr + 1) * P])
                                    else:
                                        nc.vector.tensor_copy(
                                            out=qkt_sb[:, hp * 4 + pr, nsl],
                                            in_=tp[:, pr * P:(pr + 1) * P])

                    # ladder: a3(t-4) interleaves with a1(t+4)
                    for t in range(4):
                        a1_tile(t)
                    if do_ln:
                        stats_batch(0)
                    for t in range(4, 8):
                        a1_tile(t)
                        if do_ln:
                            a3_tiles([t - 4])
                    if do_ln:
                        stats_batch(1)
                        a3_tiles(range(4, 8))

                if not do_ln or 'B' not in phases:
                    return
                # ============ phase B+C, stage-major within each query block
                with (
                    tc.tile_pool(name="pB", bufs=2) as pB,
                    tc.tile_pool(name="psB", bufs=2, space="PSUM") as psB,
                ):
                    digits = [int(ch) for ch in phases if ch.isdigit()]
                    bmax = digits[0] if digits else 9
                    for qi in range(NT):
                        qsl = slice(qi * P, (qi + 1) * P)
                        kb0 = max(qi - 2, 0)
                        nkb = qi - kb0 + 1
                        NK = nkb * P
                        den8 = pB.tile([P, HL], F32, tag="den8")
                        rec8 = pB.tile([P, HL], F32, tag="rec8")
                        p_ts, scs, ptss = [], [], []
                        # scores (PE, fp16)
                        for h in range(HL):
                            pair, poff = h // 2, (h % 2) * 64
                            sc = psB.tile([P, 3 * P], F32, tag=f"sc{h % 2}",
                                          bufs=2, name=f"sc{h}")
                            nc.tensor.matmul(
                                sc[:, 0:NK],
                                qkt_sb[poff:poff + 64, pair, qsl],
                                qkt_sb[poff:poff + 64, 4 + pair,
                                       kb0 * P:kb0 * P + NK],
                                start=True, stop=False)
                            scs.append(sc)
                        # masks via PE accumulation (identity stationary)
                        for h in range(HL):
                            nc.tensor.matmul(
                                scs[h][:, 0:NK], idh_sb,
                                mk_sb[:, 3 * P - NK:3 * P],
                                start=False, stop=True)
                        # exp (ACT): scale = y_q (rstd/8 fold), accum -> den
                        for h in range(HL):
                            p_t = pB.tile([P, 3 * P], F16, tag=f"p{h}", bufs=3,
                                          name=f"pt{h}")
                            nc.scalar.activation(
                                out=p_t[:, 0:NK], in_=scs[h][:, 0:NK],
                                func=mybir.ActivationFunctionType.Exp,
                                scale=y_g[:, qi, h:h + 1],
                                accum_out=den8[:, h:h + 1])
                            p_ts.append(p_t)
                        if bmax < 2:
                            continue
                        nc.vector.tensor_add(out=den8, in0=den8, in1=es_sb)
                        nc.vector.reciprocal(out=rec8, in_=den8)
                        # normalize p (ACT/DVE), transpose (PE), evict
                        for h in range(0 if bmax < 3 else HL):
                            p_t = p_ts[h]
                            nc.gpsimd.tensor_scalar_mul(
                                out=p_t[:, 0:NK], in0=p_t[:, 0:NK],
                                scalar1=rec8[:, h:h + 1])
                            ptp = psB.tile([P, 3, P], F16, tag=f"ptp{h % 2}",
                                           bufs=1, name=f"ptp{h}")
                            for j in range(nkb):
                                nc.tensor.transpose(
                                    ptp[:, j, :], p_t[:, j * P:(j + 1) * P],
                                    idh_sb)
                            pts = pB.tile([P, 3, P], F16, tag=f"pts{h % 2}",
                                          bufs=2, name=f"pts{h}")
                            if h % 2 == 0:
                                nc.vector.tensor_copy(out=pts[:, 0:nkb, :],
                                                      in_=ptp[:, 0:nkb, :])
                            else:
                                nc.scalar.copy(out=pts[:, 0:nkb, :],
                                               in_=ptp[:, 0:nkb, :])
                            ptss.append(pts)
                        # PV (PE) + attn evict (DVE/gpsimd)
                        at = None
                        for h in range(0 if bmax < 4 else HL):
                            pair, poff = h // 2, (h % 2) * 64
                            if h % 2 == 0:
                                at = psB.tile([P, P], F32, tag="at", bufs=1)
                            for j in range(nkb):
                                kb = kb0 + j
                                nc.tensor.matmul(
                                    at[poff:poff + 64, :],
                                    v_sb[:, kb, h * D:(h + 1) * D],
                                    ptss[h][:, j, :],
                                    start=(j == 0), stop=(j == nkb - 1))
                            if h % 2 == 1:
                                if pair % 2 == 0:
                                    nc.scalar.copy(
                                        out=att_sb[:, pair, qi, :], in_=at)
                                else:
                                    nc.vector.tensor_copy(
                                        out=att_sb[:, pair, qi, :], in_=at)
                        # ============ phase C: proj
                        for e in range(2 if 'C' in phases else 0):
                            pj_ps = psB.tile([P, 512], F32, tag="pjp", bufs=1)
                            for ch in range(4):
                                nc.tensor.matmul(
                                    pj_ps,
                                    att_sb[:, ch, qi, :],
                                    pj_sb[:, ch, e * 512:(e + 1) * 512],
                                    start=(ch == 0), stop=(ch == 3))
                            y_sb = pB.tile([P, 512], F32, tag="ysb")
                            if e == 0:
                                nc.scalar.copy(out=y_sb, in_=pj_ps)
                            else:
                                nc.vector.tensor_copy(out=y_sb, in_=pj_ps)
                            nc.sync.dma_start(
                                out=y[qsl, e * 512:(e + 1) * 512], in_=y_sb)

            if use_for_i and repeat > 1:
                with tc.For_i(0, repeat, 1):
                    body()
            else:
                for _ in range(repeat):
                    body()

    nc.finalize()
    return nc


def host_prep(x, qkv_w, qn_w, qn_b, kn_w, kn_b, sinks, proj_w, proj_b):
    """Build the 8 per-core input maps (numpy, host-side sharding + tables)."""
    f32 = np.float32
    f16 = np.float16
    n = np.arange(N, dtype=np.float64)
    inv = ROPE_BASE ** (-np.arange(0, D, 2, dtype=np.float64) / D)
    freqs = n[:, None] * inv[None, :]
    emb = np.concatenate([freqs, freqs], axis=1)
    cos, sin = np.cos(emb), np.sin(emb)
    sgn = np.concatenate([-np.ones(D // 2), np.ones(D // 2)])

    def tables(w, scale):
        w = np.asarray(w, np.float64) * scale
        w_rot = np.concatenate([w[D // 2:], w[:D // 2]])
        cw = (cos * w[None, :]).astype(f16)
        sw = (sin * w_rot[None, :] * sgn[None, :]).astype(f16)
        return np.ascontiguousarray(cw), np.ascontiguousarray(sw)

    coswq, sinwq = tables(qn_w, 1.0)
    coswk, sinwk = tables(kn_w, np.sqrt(D))  # sqrt(D) fold for k rstd
    assert np.allclose(qn_b, 0) and np.allclose(kn_b, 0), \
        "nonzero qk-norm bias not implemented"

    r = np.arange(P)[:, None]
    c = np.arange(P)[None, :]
    m_up = np.where(c > r, 0.0, NEG)   # window boundary block (kb = qi-2)
    m_lo = np.where(c <= r, 0.0, NEG)  # causal diag block (kb = qi)
    masks_np = np.ascontiguousarray(np.concatenate(
        [m_up, np.zeros((P, P)), m_lo], axis=1).astype(f16))
    identh_np = np.eye(P).astype(f16)

    def center(wrows):
        # per-head mean over d folded into weights: z' = x @ w'.T centered
        w3 = wrows.reshape(-1, D, DIM)
        return (w3 - w3.mean(axis=1, keepdims=True)).reshape(-1, DIM)

    in_maps = []
    for core in range(8):
        b, g = core // 2, core % 2
        q_rows = center(qkv_w[g * 512:(g + 1) * 512])
        k_rows = center(qkv_w[1024 + g * 512:1024 + (g + 1) * 512])
        v_rows = qkv_w[2048 + g * 512:2048 + (g + 1) * 512]
        in_maps.append({
            "xt": np.ascontiguousarray(x[b].T.astype(ml_dtypes.bfloat16)),
            "wqkt": np.ascontiguousarray(
                np.concatenate([q_rows, k_rows], 0).T.astype(ml_dtypes.bfloat16)),
            "wvt": np.ascontiguousarray(v_rows.T.astype(ml_dtypes.bfloat16)),
            "projt": np.ascontiguousarray(
                proj_w[:, g * 512:(g + 1) * 512].T.astype(f16)),
            "coswq": coswq, "sinwq": sinwq,
            "coswk": coswk, "sinwk": sinwk,
            "esink": np.exp(sinks[g * 8:(g + 1) * 8]).astype(f32).reshape(1, HL),
            "masks": masks_np,
            "identh": identh_np,
        })
    return in_maps


def assemble(results, proj_b):
    out = np.zeros((B, N, DIM), dtype=np.float32)
    for b in range(B):
        out[b] = results[2 * b]["y"] + results[2 * b + 1]["y"] + proj_b[None, :]
    return out


# ---------------------------------------------------------------------------
# Public entry point: kernel(**inputs) -> full output [B, N, DIM]
# ---------------------------------------------------------------------------
from concourse.bass_utils import run_bass_kernel_spmd

_NC_CACHE = {}


def _get_nc():
    if "nc" not in _NC_CACHE:
        _NC_CACHE["nc"] = build_nc(repeat=1)
    return _NC_CACHE["nc"]


def kernel(x, qkv_w, qn_w, qn_b, kn_w, kn_b, sinks, proj_w, proj_b):
    x = np.asarray(x, np.float32)
    qkv_w = np.asarray(qkv_w, np.float32)
    proj_w = np.asarray(proj_w, np.float32)
    in_maps = host_prep(x, qkv_w, np.asarray(qn_w), np.asarray(qn_b),
                        np.asarray(kn_w), np.asarray(kn_b),
                        np.asarray(sinks), proj_w, np.asarray(proj_b))
    nc = _get_nc()
    res = run_bass_kernel_spmd(nc, in_maps, core_ids=list(range(8)))
    return assemble(res.results, np.asarray(proj_b, np.float32))


# revision 15
# speedup vs baseline: 2.6152x; 2.6152x over previous
"""Bass/Tile kernel for sparse sliding-window attention with sinks (v2).

Problem (full): B=4, N=1024, DIM=1024, H=16, D=64, SW=256.
Sharding: 8 cores; core c -> batch b=c//2, head-group g=c%2 (8 heads each).
Host sums the two per-head-group partial projections + proj bias.

v2 changes vs baseline:
  - mean-centering folded into qkv weights on host (LN mean path removed;
    var = sumsq/D on centered z)
  - fp16 for all post-QKV tensors (zraw/qkt/v/p/att/proj weights) -> DVE
    2x/4x modes, PE 1 cyc/row
  - rstd via Quake rsqrt on DVE (no ACT Sqrt -> single act table set);
    q-side rstd folded into the softmax exp scale (free), k-side applied
    once per tile with sqrt(D) folded into the k rope tables
  - sliding-window masks applied via a PE accumulation matmul
    (identity-stationary, host mask tile moving) instead of DVE adds
  - softmax 1/den folded into the p-transpose eviction copy
  - RoPE rotate-half via strided DVE reads (no gpsimd copy)
"""

import sys

sys.path.insert(0, "/opt/trn_rl_repo")

import numpy as np
import ml_dtypes

import concourse.bass as bass
import concourse.mybir as mybir
import concourse.tile as tile
from concourse import bacc

F32 = mybir.dt.float32
F16 = mybir.dt.float16
BF16 = mybir.dt.bfloat16
I32 = mybir.dt.int32

B, N, DIM = 4, 1024, 1024
H, D = 16, 64
SW = 256
ROPE_BASE = 10000.0
LN_EPS = 1e-5
P = 128
NT = N // P      # 8 query/n tiles
CC = DIM // P    # 8 contraction chunks
HL = H // 2      # 8 local heads
NEG = -30000.0   # fp16-safe mask value
MAGIC = 0x5EF759DF  # quake magic adjusted for vh = v/2 input
PNORM = "dve"  # which engine normalizes p: pool | dve | act


def build_nc(repeat=1, use_for_i=False, phases="ABC"):
    nc = bacc.Bacc("TRN2", target_bir_lowering=False, debug=False, num_devices=8)

    xt = nc.declare_dram_parameter("xt", [DIM, N], BF16, isOutput=False)
    wqkt = nc.declare_dram_parameter("wqkt", [DIM, 1024], BF16, isOutput=False)
    wvt = nc.declare_dram_parameter("wvt", [DIM, 512], BF16, isOutput=False)
    projt = nc.declare_dram_parameter("projt", [512, DIM], F16, isOutput=False)
    coswq = nc.declare_dram_parameter("coswq", [N, D], F16, isOutput=False)
    sinwq = nc.declare_dram_parameter("sinwq", [N, D], F16, isOutput=False)
    coswk = nc.declare_dram_parameter("coswk", [N, D], F16, isOutput=False)
    sinwk = nc.declare_dram_parameter("sinwk", [N, D], F16, isOutput=False)
    esink = nc.declare_dram_parameter("esink", [1, HL], F32, isOutput=False)
    masks = nc.declare_dram_parameter("masks", [P, 3 * P], F16, isOutput=False)
    identh = nc.declare_dram_parameter("identh", [P, P], F16, isOutput=False)
    y = nc.declare_dram_parameter("y", [N, DIM], F32, isOutput=True)

    with tile.TileContext(nc) as tc:
        with tc.tile_pool(name="consts", bufs=1) as consts:
            wqk_sb = consts.tile([P, CC, 1024], BF16, tag="wqk")
            wqk_src = wqkt.ap().rearrange("(cc p) f -> p cc f", p=P)
            wv_sb = consts.tile([P, CC, 512], BF16, tag="wv")
            wv_src = wvt.ap().rearrange("(cc p) f -> p cc f", p=P)
            for c in range(CC):
                nc.sync.dma_start(out=wqk_sb[:, c, :], in_=wqk_src[:, c, :])
                nc.sync.dma_start(out=wv_sb[:, c, :], in_=wv_src[:, c, :])
            pj_sb = consts.tile([P, 4, DIM], F16, tag="pj")
            pj_src = projt.ap().rearrange("(ch p) e -> p ch e", p=P)
            for ch in range(4):
                nc.sync.dma_start(out=pj_sb[:, ch, :], in_=pj_src[:, ch, :])
            cos_sin = {}
            for nm, t_dram in (
                ("cq", coswq), ("sq", sinwq), ("ck", coswk), ("sk", sinwk),
            ):
                t_sb = consts.tile([P, NT, D], F16, tag=nm)
                nc.sync.dma_start(
                    out=t_sb, in_=t_dram.ap().rearrange("(t p) d -> p t d", p=P))
                cos_sin[nm] = t_sb
            es_sb = consts.tile([P, HL], F32, tag="es")
            nc.sync.dma_start(out=es_sb, in_=esink.ap().to_broadcast([P, HL]))
            mk_sb = consts.tile([P, 3 * P], F16, tag="mk")
            nc.sync.dma_start(out=mk_sb, in_=masks.ap())
            idh_sb = consts.tile([P, P], F16, tag="idh")
            nc.sync.dma_start(out=idh_sb, in_=identh.ap())
            magic_sb = consts.tile([P, 1], I32, tag="magic")
            nc.vector.memset(magic_sb, MAGIC)

            # persistent intermediates
            qkt_sb = consts.tile([P, 8, N], F16, tag="qkt")   # [d, pair, n]
            v_sb = consts.tile([P, NT, 512], F16, tag="v")    # [n, ktile, hd]
            att_sb = consts.tile([P, 4, NT, P], F16, tag="att")  # [hd, pair, qi, n]
            zraw = consts.tile([P, NT, 1024], F16, tag="zraw")   # centered qk
            sq_g = consts.tile([P, NT, 16], F32, tag="sqg")      # sumsq
            y_g = consts.tile([P, NT, 16], F32, tag="yg")        # rsqrt(sumsq+Deps)

            def body(phases=phases):
                with (
                    tc.tile_pool(name="pA", bufs=3) as pA,
                    tc.tile_pool(name="psA", bufs=2, space="PSUM") as psA,
                    tc.tile_pool(name="pR", bufs=1) as pR,
                ):
                    do_ln = ('L' in phases) or ('A' in phases)

                    def a1_tile(t):
                        nsl = slice(t * P, (t + 1) * P)
                        xt_t = pA.tile([P, CC, P], BF16, tag="xt", name=f"xt{t}")
                        nc.sync.dma_start(
                            out=xt_t,
                            in_=xt[:, nsl].rearrange("(cc p) n -> p cc n", p=P))
                        ps_q = psA.tile([P, 512], F32, tag="psq", name=f"psq{t}")
                        ps_k = psA.tile([P, 512], F32, tag="psk", name=f"psk{t}")
                        ps_v = psA.tile([P, 512], F32, tag="psv", name=f"psv{t}")
                        for c in range(CC):
                            st, sp = (c == 0), (c == CC - 1)
                            nc.tensor.matmul(ps_q, xt_t[:, c, :], wqk_sb[:, c, 0:512],
                                             start=st, stop=sp)
                            nc.tensor.matmul(ps_k, xt_t[:, c, :], wqk_sb[:, c, 512:1024],
                                             start=st, stop=sp)
                            nc.tensor.matmul(ps_v, xt_t[:, c, :], wv_sb[:, c, :],
                                             start=st, stop=sp)
                        nc.scalar.copy(out=v_sb[:, t, :], in_=ps_v)
                        if not do_ln:
                            return
                        # evict centered z to fp16 (ACT), sumsq via DVE on fp16
                        nc.scalar.copy(out=zraw[:, t, 0:512], in_=ps_q)
                        nc.scalar.copy(out=zraw[:, t, 512:1024], in_=ps_k)
                        sq = pR.tile([P, 16, D], F16, tag=f"sq{t % 2}",
                                     name=f"sq{t}")
                        z16 = zraw[:, t, :].rearrange("p (s d) -> p s d", d=D)
                        nc.vector.tensor_mul(out=sq, in0=z16, in1=z16)
                        with nc.allow_low_precision("fp16 sumsq ok for rstd"):
                            nc.vector.tensor_reduce(
                                out=sq_g[:, t, :], in_=sq,
                                axis=mybir.AxisListType.X, op=mybir.AluOpType.add)

                    def stats_batch(g):
                        # y = rsqrt(sumsq + D*eps) for tiles [2g, 2g+2) via
                        # quake rsqrt + 2 newton steps (all DVE, no ACT table)
                        tsl = slice(g * 2, g * 2 + 2)
                        ysl = y_g[:, tsl, :]
                        vh = pR.tile([P, 2, 16], F32, tag=f"vh{g % 2}", name=f"vh{g}")
                        yt = pR.tile([P, 2, 16], F32, tag=f"yt{g % 2}", name=f"yt{g}")
                        sc2 = pR.tile([P, 2, 16], F32, tag=f"sc2{g % 2}", name=f"sc2{g}")
                        nc.vector.tensor_scalar(
                            out=vh, in0=sq_g[:, tsl, :], scalar1=0.5,
                            scalar2=0.5 * D * LN_EPS,
                            op0=mybir.AluOpType.mult, op1=mybir.AluOpType.add)
                        ivh = vh.bitcast(I32)
                        iy = yt.bitcast(I32)
                        nc.vector.tensor_scalar(
                            out=iy, in0=ivh, scalar1=1, scalar2=None,
                            op0=mybir.AluOpType.logical_shift_right)
                        nc.vector.tensor_tensor(
                            out=iy, in0=magic_sb.broadcast_to([P, 2, 16]),
                            in1=iy, op=mybir.AluOpType.subtract)
                        for it in range(2):
                            dst = ysl if it == 1 else yt
                            nc.vector.tensor_mul(out=sc2, in0=yt, in1=yt)
                            nc.vector.tensor_mul(out=sc2, in0=sc2, in1=vh)
                            nc.vector.tensor_scalar(
                                out=sc2, in0=sc2, scalar1=-1.0, scalar2=1.5,
                                op0=mybir.AluOpType.mult, op1=mybir.AluOpType.add)
                            nc.vector.tensor_mul(out=dst, in0=yt, in1=sc2)

                    def a3_tiles(ts):
                        for t in ts:
                            rot = pR.tile([P, 1024], F16, tag=f"rot{t % 2}",
                                          name=f"rotb{t}")
                            tmp = pR.tile([P, 1024], F16, tag=f"tmp{t % 2}",
                                          name=f"tmpb{t}")
                            J = D // 2
                            zv = zraw[:, t, :].rearrange(
                                "p (s h j) -> p s h j", h=2, j=J)
                            rv = rot.rearrange("p (s h j) -> p s h j", h=2, j=J)
                            tv = tmp.rearrange("p (s d) -> p s d", d=D)
                            for half, (cn, sn) in enumerate(
                                    (("cq", "sq"), ("ck", "sk"))):
                                ssl = slice(half * 8, half * 8 + 8)
                                cb = cos_sin[cn][:, t, :].unsqueeze(1)\
                                    .broadcast_to([P, 8, D])
                                sb3 = cos_sin[sn][:, t, :]\
                                    .rearrange("p (h j) -> p h j", j=J)
                                # t = z * cos
                                nc.vector.tensor_mul(
                                    out=tv[:, ssl, :],
                                    in0=zraw[:, t, half * 512:(half + 1) * 512]
                                    .rearrange("p (s d) -> p s d", d=D),
                                    in1=cb)
                                # r[j'] = z[1-j'] * sin[j']  (rotate-half)
                                for jp in range(2):
                                    nc.vector.tensor_mul(
                                        out=rv[:, ssl, jp, :],
                                        in0=zv[:, ssl, 1 - jp, :],
                                        in1=sb3[:, jp, :].unsqueeze(1)
                                        .broadcast_to([P, 8, J]))
                            # u = t + r, k-half scaled by y_k after
                            nc.vector.tensor_add(
                                out=zraw[:, t, :], in0=tmp, in1=rot)
                            # k *= y_k (8/sqrt fold in tables)
                            zk = zraw[:, t, 512:1024].rearrange(
                                "p (s d) -> p s d", d=D)
                            nc.vector.tensor_mul(
                                out=zk, in0=zk,
                                in1=y_g[:, t, 8:16].unsqueeze(2)
                                .broadcast_to([P, 8, D]))
                        for t in ts:
                            nsl = slice(t * P, (t + 1) * P)
                            for hp in range(2):  # q half / k half
                                tp = psA.tile([P, 512], F16, tag="tp",
                                              name=f"tp{t}_{hp}")
                                for pr in range(4):
                                    blk = hp * 4 + pr
                                    nc.tensor.transpose(
                                        tp[:, pr * P:(pr + 1) * P],
                                        zraw[:, t, blk * P:(blk + 1) * P],
                                        idh_sb)
                                # evict 4 transposed blocks to qkt
                                for pr in range(4):
                                    if hp == 0:
                                        nc.scalar.copy(
                                            out=qkt_sb[:, hp * 4 + pr, nsl],
                                            in_=tp[:, pr * P:(pr + 1) * P])
                                    else:
                                        nc.vector.tensor_copy(
                                            out=qkt_sb[:, hp * 4 + pr, nsl],
                                            in_=tp[:, pr * P:(pr + 1) * P])

                    # ladder: grain-2 stats, a3 lags a1 by 3 tiles
                    for t in range(8):
                        a1_tile(t)
                        if not do_ln:
                            continue
                        if t % 2 == 1:
                            stats_batch(t // 2)
                        if t >= 3:
                            a3_tiles([t - 3])
                    if do_ln:
                        a3_tiles(range(5, 8))

                if not do_ln or 'B' not in phases:
                    return
                # ============ phase B+C, stage-major within each query block
                with (
                    tc.tile_pool(name="pB", bufs=2) as pB,
                    tc.tile_pool(name="psB", bufs=2, space="PSUM") as psB,
                ):
                    digits = [int(ch) for ch in phases if ch.isdigit()]
                    bmax = digits[0] if digits else 9
                    for qi in range(NT):
                        qsl = slice(qi * P, (qi + 1) * P)
                        kb0 = max(qi - 2, 0)
                        nkb = qi - kb0 + 1
                        NK = nkb * P
                        den8 = pB.tile([P, HL], F32, tag="den8")
                        rec8 = pB.tile([P, HL], F32, tag="rec8")
                        p_ts, scs, ptss = [], [], []
                        # scores (PE, fp16)
                        for h in range(HL):
                            pair, poff = h // 2, (h % 2) * 64
                            sc = psB.tile([P, 3 * P], F32, tag=f"sc{h % 2}",
                                          bufs=2, name=f"sc{h}")
                            nc.tensor.matmul(
                                sc[:, 0:NK],
                                qkt_sb[poff:poff + 64, pair, qsl],
                                qkt_sb[poff:poff + 64, 4 + pair,
                                       kb0 * P:kb0 * P + NK],
                                start=True, stop=False)
                            scs.append(sc)
                        # masks via PE accumulation (identity stationary)
                        for h in range(HL):
                            nc.tensor.matmul(
                                scs[h][:, 0:NK], idh_sb,
                                mk_sb[:, 3 * P - NK:3 * P],
                                start=False, stop=True)
                        # exp (ACT): scale = y_q (rstd/8 fold), accum -> den
                        for h in range(HL):
                            p_t = pB.tile([P, 3 * P], F16, tag=f"p{h}", bufs=3,
                                          name=f"pt{h}")
                            nc.scalar.activation(
                                out=p_t[:, 0:NK], in_=scs[h][:, 0:NK],
                                func=mybir.ActivationFunctionType.Exp,
                                scale=y_g[:, qi, h:h + 1],
                                accum_out=den8[:, h:h + 1])
                            p_ts.append(p_t)
                        if bmax < 2:
                            continue
                        nc.vector.tensor_add(out=den8, in0=den8, in1=es_sb)
                        nc.vector.reciprocal(out=rec8, in_=den8)
                        # normalize p (ACT/DVE), transpose (PE), evict
                        for h in range(0 if bmax < 3 else HL):
                            p_t = p_ts[h]
                            if PNORM == "pool":
                                nc.gpsimd.tensor_scalar_mul(
                                    out=p_t[:, 0:NK], in0=p_t[:, 0:NK],
                                    scalar1=rec8[:, h:h + 1])
                            elif PNORM == "dve":
                                nc.vector.tensor_scalar_mul(
                                    out=p_t[:, 0:NK], in0=p_t[:, 0:NK],
                                    scalar1=rec8[:, h:h + 1])
                            else:
                                nc.scalar.activation(
                                    out=p_t[:, 0:NK], in_=p_t[:, 0:NK],
                                    func=mybir.ActivationFunctionType.Identity,
                                    scale=rec8[:, h:h + 1])
                            ptp = psB.tile([P, 3, P], F16, tag=f"ptp{h % 2}",
                                           bufs=1, name=f"ptp{h}")
                            for j in range(nkb):
                                nc.tensor.transpose(
                                    ptp[:, j, :], p_t[:, j * P:(j + 1) * P],
                                    idh_sb)
                            pts = pB.tile([P, 3, P], F16, tag=f"pts{h % 2}",
                                          bufs=2, name=f"pts{h}")
                            if h % 2 == 0:
                                nc.vector.tensor_copy(out=pts[:, 0:nkb, :],
                                                      in_=ptp[:, 0:nkb, :])
                            else:
                                nc.scalar.copy(out=pts[:, 0:nkb, :],
                                               in_=ptp[:, 0:nkb, :])
                            ptss.append(pts)
                        # PV (PE) + attn evict (DVE/gpsimd)
                        at = None
                        for h in range(0 if bmax < 4 else HL):
                            pair, poff = h // 2, (h % 2) * 64
                            if h % 2 == 0:
                                at = psB.tile([P, P], F32, tag="at", bufs=1)
                            for j in range(nkb):
                                kb = kb0 + j
                                nc.tensor.matmul(
                                    at[poff:poff + 64, :],
                                    v_sb[:, kb, h * D:(h + 1) * D],
                                    ptss[h][:, j, :],
                                    start=(j == 0), stop=(j == nkb - 1))
                            if h % 2 == 1:
                                if pair % 2 == 0:
                                    nc.scalar.copy(
                                        out=att_sb[:, pair, qi, :], in_=at)
                                else:
                                    nc.vector.tensor_copy(
                                        out=att_sb[:, pair, qi, :], in_=at)
                        # ============ phase C: proj
                        for e in range(2 if 'C' in phases else 0):
                            pj_ps = psB.tile([P, 512], F32, tag="pjp", bufs=1)
                            for ch in range(4):
                                nc.tensor.matmul(
                                    pj_ps,
                                    att_sb[:, ch, qi, :],
                                    pj_sb[:, ch, e * 512:(e + 1) * 512],
                                    start=(ch == 0), stop=(ch == 3))
                            y_sb = pB.tile([P, 512], F32, tag="ysb")
                            if e == 0:
                                nc.scalar.copy(out=y_sb, in_=pj_ps)
                            else:
                                nc.vector.tensor_copy(out=y_sb, in_=pj_ps)
                            nc.sync.dma_start(
                                out=y[qsl, e * 512:(e + 1) * 512], in_=y_sb)

            if use_for_i and repeat > 1:
                with tc.For_i(0, repeat, 1):
                    body()
            else:
                for _ in range(repeat):
                    body()

    nc.finalize()
    return nc


def host_prep(x, qkv_w, qn_w, qn_b, kn_w, kn_b, sinks, proj_w, proj_b):
    """Build the 8 per-core input maps (numpy, host-side sharding + tables)."""
    f32 = np.float32
    f16 = np.float16
    n = np.arange(N, dtype=np.float64)
    inv = ROPE_BASE ** (-np.arange(0, D, 2, dtype=np.float64) / D)
    freqs = n[:, None] * inv[None, :]
    emb = np.concatenate([freqs, freqs], axis=1)
    cos, sin = np.cos(emb), np.sin(emb)
    sgn = np.concatenate([-np.ones(D // 2), np.ones(D // 2)])

    def tables(w, scale):
        w = np.asarray(w, np.float64) * scale
        w_rot = np.concatenate([w[D // 2:], w[:D // 2]])
        cw = (cos * w[None, :]).astype(f16)
        sw = (sin * w_rot[None, :] * sgn[None, :]).astype(f16)
        return np.ascontiguousarray(cw), np.ascontiguousarray(sw)

    coswq, sinwq = tables(qn_w, 1.0)
    coswk, sinwk = tables(kn_w, np.sqrt(D))  # sqrt(D) fold for k rstd
    assert np.allclose(qn_b, 0) and np.allclose(kn_b, 0), \
        "nonzero qk-norm bias not implemented"

    r = np.arange(P)[:, None]
    c = np.arange(P)[None, :]
    m_up = np.where(c > r, 0.0, NEG)   # window boundary block (kb = qi-2)
    m_lo = np.where(c <= r, 0.0, NEG)  # causal diag block (kb = qi)
    masks_np = np.ascontiguousarray(np.concatenate(
        [m_up, np.zeros((P, P)), m_lo], axis=1).astype(f16))
    identh_np = np.eye(P).astype(f16)

    def center(wrows):
        # per-head mean over d folded into weights: z' = x @ w'.T centered
        w3 = wrows.reshape(-1, D, DIM)
        return (w3 - w3.mean(axis=1, keepdims=True)).reshape(-1, DIM)

    in_maps = []
    for core in range(8):
        b, g = core // 2, core % 2
        q_rows = center(qkv_w[g * 512:(g + 1) * 512])
        k_rows = center(qkv_w[1024 + g * 512:1024 + (g + 1) * 512])
        v_rows = qkv_w[2048 + g * 512:2048 + (g + 1) * 512]
        in_maps.append({
            "xt": np.ascontiguousarray(x[b].T.astype(ml_dtypes.bfloat16)),
            "wqkt": np.ascontiguousarray(
                np.concatenate([q_rows, k_rows], 0).T.astype(ml_dtypes.bfloat16)),
            "wvt": np.ascontiguousarray(v_rows.T.astype(ml_dtypes.bfloat16)),
            "projt": np.ascontiguousarray(
                proj_w[:, g * 512:(g + 1) * 512].T.astype(f16)),
            "coswq": coswq, "sinwq": sinwq,
            "coswk": coswk, "sinwk": sinwk,
            "esink": np.exp(sinks[g * 8:(g + 1) * 8]).astype(f32).reshape(1, HL),
            "masks": masks_np,
            "identh": identh_np,
        })
    return in_maps


def assemble(results, proj_b):
    out = np.zeros((B, N, DIM), dtype=np.float32)
    for b in range(B):
        out[b] = results[2 * b]["y"] + results[2 * b + 1]["y"] + proj_b[None, :]
    return out


# ---------------------------------------------------------------------------
# Public entry point: kernel(**inputs) -> full output [B, N, DIM]
# ---------------------------------------------------------------------------
from concourse.bass_utils import run_bass_kernel_spmd

_NC_CACHE = {}


def _get_nc():
    if "nc" not in _NC_CACHE:
        _NC_CACHE["nc"] = build_nc(repeat=1)
    return _NC_CACHE["nc"]


def kernel(x, qkv_w, qn_w, qn_b, kn_w, kn_b, sinks, proj_w, proj_b):
    x = np.asarray(x, np.float32)
    qkv_w = np.asarray(qkv_w, np.float32)
    proj_w = np.asarray(proj_w, np.float32)
    in_maps = host_prep(x, qkv_w, np.asarray(qn_w), np.asarray(qn_b),
                        np.asarray(kn_w), np.asarray(kn_b),
                        np.asarray(sinks), proj_w, np.asarray(proj_b))
    nc = _get_nc()
    res = run_bass_kernel_spmd(nc, in_maps, core_ids=list(range(8)))
    return assemble(res.results, np.asarray(proj_b, np.float32))


# revision 21
# speedup vs baseline: 2.6698x; 1.0209x over previous
"""Bass/Tile kernel for sparse sliding-window attention with sinks (v2).

Problem (full): B=4, N=1024, DIM=1024, H=16, D=64, SW=256.
Sharding: 8 cores; core c -> batch b=c//2, head-group g=c%2 (8 heads each).
Host sums the two per-head-group partial projections + proj bias.

v2 changes vs baseline:
  - mean-centering folded into qkv weights on host (LN mean path removed;
    var = sumsq/D on centered z)
  - fp16 for all post-QKV tensors (zraw/qkt/v/p/att/proj weights) -> DVE
    2x/4x modes, PE 1 cyc/row
  - rstd via Quake rsqrt on DVE (no ACT Sqrt -> single act table set);
    q-side rstd folded into the softmax exp scale (free), k-side applied
    once per tile with sqrt(D) folded into the k rope tables
  - sliding-window masks applied via a PE accumulation matmul
    (identity-stationary, host mask tile moving) instead of DVE adds
  - softmax 1/den folded into the p-transpose eviction copy
  - RoPE rotate-half via strided DVE reads (no gpsimd copy)
"""

import sys

sys.path.insert(0, "/opt/trn_rl_repo")

import numpy as np
import ml_dtypes

import concourse.bass as bass
import concourse.mybir as mybir
import concourse.tile as tile
from concourse import bacc

F32 = mybir.dt.float32
F16 = mybir.dt.float16
BF16 = mybir.dt.bfloat16
I32 = mybir.dt.int32

B, N, DIM = 4, 1024, 1024
H, D = 16, 64
SW = 256
ROPE_BASE = 10000.0
LN_EPS = 1e-5
P = 128
NT = N // P      # 8 query/n tiles
CC = DIM // P    # 8 contraction chunks
HL = H // 2      # 8 local heads
NEG = -30000.0   # fp16-safe mask value
MAGIC = 0x5EF759DF  # quake magic adjusted for vh = v/2 input
PNORM = "dve"  # which engine normalizes p: pool | dve | act


def build_nc(repeat=1, use_for_i=False, phases="ABC"):
    nc = bacc.Bacc("TRN2", target_bir_lowering=False, debug=False, num_devices=8)

    xt = nc.declare_dram_parameter("xt", [DIM, N], BF16, isOutput=False)
    wqkt = nc.declare_dram_parameter("wqkt", [DIM, 1024], BF16, isOutput=False)
    wvt = nc.declare_dram_parameter("wvt", [DIM, 512], BF16, isOutput=False)
    projt = nc.declare_dram_parameter("projt", [512, DIM], F16, isOutput=False)
    coswq = nc.declare_dram_parameter("coswq", [N, D], F16, isOutput=False)
    sinwq = nc.declare_dram_parameter("sinwq", [N, D], F16, isOutput=False)
    coswk = nc.declare_dram_parameter("coswk", [N, D], F16, isOutput=False)
    sinwk = nc.declare_dram_parameter("sinwk", [N, D], F16, isOutput=False)
    esink = nc.declare_dram_parameter("esink", [1, HL], F32, isOutput=False)
    masks = nc.declare_dram_parameter("masks", [P, 3 * P], F16, isOutput=False)
    identh = nc.declare_dram_parameter("identh", [P, P], F16, isOutput=False)
    y = nc.declare_dram_parameter("y", [N, DIM], F32, isOutput=True)

    with tile.TileContext(nc) as tc:
        with tc.tile_pool(name="consts", bufs=1) as consts:
            wqk_sb = consts.tile([P, CC, 1024], BF16, tag="wqk")
            wqk_src = wqkt.ap().rearrange("(cc p) f -> p cc f", p=P)
            wv_sb = consts.tile([P, CC, 512], BF16, tag="wv")
            wv_src = wvt.ap().rearrange("(cc p) f -> p cc f", p=P)
            for c in range(CC):
                nc.sync.dma_start(out=wqk_sb[:, c, :], in_=wqk_src[:, c, :])
                nc.sync.dma_start(out=wv_sb[:, c, :], in_=wv_src[:, c, :])
            pj_sb = consts.tile([P, 4, DIM], F16, tag="pj")
            pj_src = projt.ap().rearrange("(ch p) e -> p ch e", p=P)
            for ch in range(4):
                nc.sync.dma_start(out=pj_sb[:, ch, :], in_=pj_src[:, ch, :])
            cos_sin = {}
            for nm, t_dram in (
                ("cq", coswq), ("sq", sinwq), ("ck", coswk), ("sk", sinwk),
            ):
                t_sb = consts.tile([P, NT, D], F16, tag=nm)
                nc.sync.dma_start(
                    out=t_sb, in_=t_dram.ap().rearrange("(t p) d -> p t d", p=P))
                cos_sin[nm] = t_sb
            es_sb = consts.tile([P, HL], F32, tag="es")
            nc.sync.dma_start(out=es_sb, in_=esink.ap().to_broadcast([P, HL]))
            mk_sb = consts.tile([P, 3 * P], F16, tag="mk")
            nc.sync.dma_start(out=mk_sb, in_=masks.ap())
            idh_sb = consts.tile([P, P], F16, tag="idh")
            nc.sync.dma_start(out=idh_sb, in_=identh.ap())
            magic_sb = consts.tile([P, 1], I32, tag="magic")
            nc.vector.memset(magic_sb, MAGIC)

            # persistent intermediates
            qkt_sb = consts.tile([P, 8, N], F16, tag="qkt")   # [d, pair, n]
            v_sb = consts.tile([P, NT, 512], F16, tag="v")    # [n, ktile, hd]
            att_sb = consts.tile([P, 4, NT, P], F16, tag="att")  # [hd, pair, qi, n]
            zraw = consts.tile([P, NT, 1024], F16, tag="zraw")   # centered qk
            sq_g = consts.tile([P, NT, 16], F32, tag="sqg")      # sumsq
            y_g = consts.tile([P, NT, 16], F32, tag="yg")        # rsqrt(sumsq+Deps)

            def body(phases=phases):
                with (
                    tc.tile_pool(name="pA", bufs=3) as pA,
                    tc.tile_pool(name="psA", bufs=2, space="PSUM") as psA,
                    tc.tile_pool(name="pR", bufs=1) as pR,
                    tc.tile_pool(name="pB", bufs=2) as pB,
                ):
                    psB = psA
                    do_ln = ('L' in phases) or ('A' in phases)

                    def a1_tile(t):
                        nsl = slice(t * P, (t + 1) * P)
                        xt_t = pA.tile([P, CC, P], BF16, tag="xt", name=f"xt{t}")
                        nc.sync.dma_start(
                            out=xt_t,
                            in_=xt[:, nsl].rearrange("(cc p) n -> p cc n", p=P))
                        ps_q = psA.tile([P, 512], F32, tag="psq", name=f"psq{t}")
                        ps_k = psA.tile([P, 512], F32, tag="psk", name=f"psk{t}")
                        ps_v = psA.tile([P, 512], F32, tag="psv", name=f"psv{t}")
                        for c in range(CC):
                            st, sp = (c == 0), (c == CC - 1)
                            nc.tensor.matmul(ps_q, xt_t[:, c, :], wqk_sb[:, c, 0:512],
                                             start=st, stop=sp)
                            nc.tensor.matmul(ps_k, xt_t[:, c, :], wqk_sb[:, c, 512:1024],
                                             start=st, stop=sp)
                            nc.tensor.matmul(ps_v, xt_t[:, c, :], wv_sb[:, c, :],
                                             start=st, stop=sp)
                        nc.scalar.copy(out=v_sb[:, t, :], in_=ps_v)
                        if not do_ln:
                            return
                        # evict centered z to fp16 (ACT), sumsq via DVE on fp16
                        nc.scalar.copy(out=zraw[:, t, 0:512], in_=ps_q)
                        nc.scalar.copy(out=zraw[:, t, 512:1024], in_=ps_k)
                        sq = pR.tile([P, 16, D], F16, tag=f"sq{t % 2}",
                                     name=f"sq{t}")
                        z16 = zraw[:, t, :].rearrange("p (s d) -> p s d", d=D)
                        nc.vector.tensor_mul(out=sq, in0=z16, in1=z16)
                        with nc.allow_low_precision("fp16 sumsq ok for rstd"):
                            nc.vector.tensor_reduce(
                                out=sq_g[:, t, :], in_=sq,
                                axis=mybir.AxisListType.X, op=mybir.AluOpType.add)

                    def stats_batch(g):
                        # y = rsqrt(sumsq + D*eps) for tiles [2g, 2g+2) via
                        # quake rsqrt + 2 newton steps (all DVE, no ACT table)
                        tsl = slice(g * 2, g * 2 + 2)
                        ysl = y_g[:, tsl, :]
                        vh = pR.tile([P, 2, 16], F32, tag=f"vh{g % 2}", name=f"vh{g}")
                        yt = pR.tile([P, 2, 16], F32, tag=f"yt{g % 2}", name=f"yt{g}")
                        sc2 = pR.tile([P, 2, 16], F32, tag=f"sc2{g % 2}", name=f"sc2{g}")
                        nc.vector.tensor_scalar(
                            out=vh, in0=sq_g[:, tsl, :], scalar1=0.5,
                            scalar2=0.5 * D * LN_EPS,
                            op0=mybir.AluOpType.mult, op1=mybir.AluOpType.add)
                        ivh = vh.bitcast(I32)
                        iy = yt.bitcast(I32)
                        nc.vector.tensor_scalar(
                            out=iy, in0=ivh, scalar1=1, scalar2=None,
                            op0=mybir.AluOpType.logical_shift_right)
                        nc.vector.tensor_tensor(
                            out=iy, in0=magic_sb.broadcast_to([P, 2, 16]),
                            in1=iy, op=mybir.AluOpType.subtract)
                        for it in range(2):
                            dst = ysl if it == 1 else yt
                            nc.vector.tensor_mul(out=sc2, in0=yt, in1=yt)
                            nc.vector.tensor_mul(out=sc2, in0=sc2, in1=vh)
                            nc.vector.tensor_scalar(
                                out=sc2, in0=sc2, scalar1=-1.0, scalar2=1.5,
                                op0=mybir.AluOpType.mult, op1=mybir.AluOpType.add)
                            nc.vector.tensor_mul(out=dst, in0=yt, in1=sc2)

                    def a3_tiles(ts):
                        for t in ts:
                            rot = pR.tile([P, 1024], F16, tag=f"rot{t % 2}",
                                          name=f"rotb{t}")
                            tmp = pR.tile([P, 1024], F16, tag=f"tmp{t % 2}",
                                          name=f"tmpb{t}")
                            J = D // 2
                            zv = zraw[:, t, :].rearrange(
                                "p (s h j) -> p s h j", h=2, j=J)
                            rv = rot.rearrange("p (s h j) -> p s h j", h=2, j=J)
                            tv = tmp.rearrange("p (s d) -> p s d", d=D)
                            for half, (cn, sn) in enumerate(
                                    (("cq", "sq"), ("ck", "sk"))):
                                ssl = slice(half * 8, half * 8 + 8)
                                cb = cos_sin[cn][:, t, :].unsqueeze(1)\
                                    .broadcast_to([P, 8, D])
                                sb3 = cos_sin[sn][:, t, :]\
                                    .rearrange("p (h j) -> p h j", j=J)
                                # t = z * cos
                                nc.vector.tensor_mul(
                                    out=tv[:, ssl, :],
                                    in0=zraw[:, t, half * 512:(half + 1) * 512]
                                    .rearrange("p (s d) -> p s d", d=D),
                                    in1=cb)
                                # r[j'] = z[1-j'] * sin[j']  (rotate-half)
                                for jp in range(2):
                                    nc.vector.tensor_mul(
                                        out=rv[:, ssl, jp, :],
                                        in0=zv[:, ssl, 1 - jp, :],
                                        in1=sb3[:, jp, :].unsqueeze(1)
                                        .broadcast_to([P, 8, J]))
                            # u = t + r, k-half scaled by y_k after
                            nc.vector.tensor_add(
                                out=zraw[:, t, :], in0=tmp, in1=rot)
                            # k *= y_k (8/sqrt fold in tables)
                            zk = zraw[:, t, 512:1024].rearrange(
                                "p (s d) -> p s d", d=D)
                            nc.vector.tensor_mul(
                                out=zk, in0=zk,
                                in1=y_g[:, t, 8:16].unsqueeze(2)
                                .broadcast_to([P, 8, D]))
                        for t in ts:
                            nsl = slice(t * P, (t + 1) * P)
                            for hp in range(2):  # q half / k half
                                tp = psA.tile([P, 512], F16, tag="tp",
                                              name=f"tp{t}_{hp}")
                                for pr in range(4):
                                    blk = hp * 4 + pr
                                    nc.tensor.transpose(
                                        tp[:, pr * P:(pr + 1) * P],
                                        zraw[:, t, blk * P:(blk + 1) * P],
                                        idh_sb)
                                # evict 4 transposed blocks to qkt
                                for pr in range(4):
                                    if hp == 0:
                                        nc.scalar.copy(
                                            out=qkt_sb[:, hp * 4 + pr, nsl],
                                            in_=tp[:, pr * P:(pr + 1) * P])
                                    else:
                                        nc.vector.tensor_copy(
                                            out=qkt_sb[:, hp * 4 + pr, nsl],
                                            in_=tp[:, pr * P:(pr + 1) * P])

                    # ladder: grain-2 stats; a3(t-3) emitted before a1(t)
                    for t in range(8):
                        if do_ln and t >= 3:
                            a3_tiles([t - 3])
                        a1_tile(t)
                        if do_ln and t % 2 == 1:
                            stats_batch(t // 2)
                    if do_ln:
                        a3_tiles(range(5, 8))

                    if not do_ln or 'B' not in phases:
                        return
                    # ======== phase B+C, software-pipelined over qi
                    digits = [int(ch) for ch in phases if ch.isdigit()]
                    bmax = digits[0] if digits else 9
                    def stage1(qi):
                        qsl = slice(qi * P, (qi + 1) * P)
                        kb0 = max(qi - 2, 0)
                        nkb = qi - kb0 + 1
                        NK = nkb * P
                        den8 = pB.tile([P, HL], F32, tag="den8")
                        rec8 = pB.tile([P, HL], F32, tag="rec8")
                        p_ts, scs = [], []
                        # scores (PE, fp16)
                        for h in range(HL):
                            pair, poff = h // 2, (h % 2) * 64
                            sc = psB.tile([P, 512], F32,
                                          tag="psq" if h % 2 == 0 else "psk",
                                          name=f"sc{h}")[:, 0:3 * P]
                            nc.tensor.matmul(
                                sc[:, 0:NK],
                                qkt_sb[poff:poff + 64, pair, qsl],
                                qkt_sb[poff:poff + 64, 4 + pair,
                                       kb0 * P:kb0 * P + NK],
                                start=True, stop=True)
                            scs.append(sc)
                        # masks via PE accumulation (identity stationary)
                        for h in range(HL):
                            sc = scs[h]
                            if nkb < 3:
                                nc.tensor.matmul(
                                    sc[:, (nkb - 1) * P:NK], idh_sb,
                                    mk_sb[:, 2 * P:3 * P],
                                    start=False, stop=True,
                                    skip_group_check=True)
                            else:
                                nc.tensor.matmul(
                                    sc[:, 0:P], idh_sb, mk_sb[:, 0:P],
                                    start=False, stop=True,
                                    skip_group_check=True)
                                nc.tensor.matmul(
                                    sc[:, 2 * P:NK], idh_sb,
                                    mk_sb[:, 2 * P:3 * P],
                                    start=False, stop=True,
                                    skip_group_check=True)
                        # exp (ACT): scale = y_q, accum -> den
                        for h in range(HL):
                            p_t = pB.tile([P, 3 * P], F16, tag=f"p{h}", bufs=3,
                                          name=f"pt{h}")
                            nc.scalar.activation(
                                out=p_t[:, 0:NK], in_=scs[h][:, 0:NK],
                                func=mybir.ActivationFunctionType.Exp,
                                scale=y_g[:, qi, h:h + 1],
                                accum_out=den8[:, h:h + 1])
                            p_ts.append(p_t)
                        if bmax < 2:
                            return None
                        nc.vector.tensor_add(out=den8, in0=den8, in1=es_sb)
                        nc.vector.reciprocal(out=rec8, in_=den8)
                        # normalize p (DVE/ACT)
                        for h in range(0 if bmax < 3 else HL):
                            p_t = p_ts[h]
                            if PNORM == "dve":
                                nc.vector.tensor_scalar_mul(
                                    out=p_t[:, 0:NK], in0=p_t[:, 0:NK],
                                    scalar1=rec8[:, h:h + 1])
                            elif PNORM == "pool":
                                nc.gpsimd.tensor_scalar_mul(
                                    out=p_t[:, 0:NK], in0=p_t[:, 0:NK],
                                    scalar1=rec8[:, h:h + 1])
                            else:
                                nc.scalar.activation(
                                    out=p_t[:, 0:NK], in_=p_t[:, 0:NK],
                                    func=mybir.ActivationFunctionType.Identity,
                                    scale=rec8[:, h:h + 1])
                        return p_ts

                    def stage2(qi, p_ts):
                        qsl = slice(qi * P, (qi + 1) * P)
                        kb0 = max(qi - 2, 0)
                        nkb = qi - kb0 + 1
                        ptss = []
                        # transpose p (PE) + evict (DVE/ACT)
                        for h in range(0 if bmax < 3 else HL):
                            p_t = p_ts[h]
                            ptp = psB.tile([P, 512], F16, tag="tp",
                                           name=f"ptp{h}")\
                                .rearrange("p (j q) -> p j q", q=P)
                            for j in range(nkb):
                                nc.tensor.transpose(
                                    ptp[:, j, :], p_t[:, j * P:(j + 1) * P],
                                    idh_sb)
                            pts = pB.tile([P, 3, P], F16, tag=f"pts{h % 2}",
                                          bufs=2, name=f"pts{h}")
                            if h % 2 == 0:
                                nc.vector.tensor_copy(out=pts[:, 0:nkb, :],
                                                      in_=ptp[:, 0:nkb, :])
                            else:
                                nc.scalar.copy(out=pts[:, 0:nkb, :],
                                               in_=ptp[:, 0:nkb, :])
                            ptss.append(pts)
                        # PV (PE) + attn evict (ACT/DVE)
                        at = None
                        for h in range(0 if bmax < 4 else HL):
                            pair, poff = h // 2, (h % 2) * 64
                            if h % 2 == 0:
                                at = psB.tile([P, 512], F32, tag="psv",
                                              name=f"at{h}")[:, 0:P]
                            for j in range(nkb):
                                kb = kb0 + j
                                nc.tensor.matmul(
                                    at[poff:poff + 64, :],
                                    v_sb[:, kb, h * D:(h + 1) * D],
                                    ptss[h][:, j, :],
                                    start=(j == 0), stop=(j == nkb - 1))
                            if h % 2 == 1:
                                if pair % 2 == 0:
                                    nc.scalar.copy(
                                        out=att_sb[:, pair, qi, :], in_=at)
                                else:
                                    nc.vector.tensor_copy(
                                        out=att_sb[:, pair, qi, :], in_=at)
                        # ============ phase C: proj
                        for e in range(2 if 'C' in phases else 0):
                            pj_ps = psB.tile([P, 512], F32, tag="psv",
                                             name=f"pj{e}")
                            for ch in range(4):
                                nc.tensor.matmul(
                                    pj_ps,
                                    att_sb[:, ch, qi, :],
                                    pj_sb[:, ch, e * 512:(e + 1) * 512],
                                    start=(ch == 0), stop=(ch == 3))
                            y_sb = pB.tile([P, 512], F32, tag="ysb")
                            if e == 0:
                                nc.scalar.copy(out=y_sb, in_=pj_ps)
                            else:
                                nc.vector.tensor_copy(out=y_sb, in_=pj_ps)
                            nc.sync.dma_start(
                                out=y[qsl, e * 512:(e + 1) * 512], in_=y_sb)

                    prev = None
                    for qi in range(NT):
                        p_ts = stage1(qi)
                        if prev is not None:
                            stage2(qi - 1, prev)
                        prev = p_ts
                    if prev is not None:
                        stage2(NT - 1, prev)

            if use_for_i and repeat > 1:
                with tc.For_i(0, repeat, 1):
                    body()
            else:
                for _ in range(repeat):
                    body()

    nc.finalize()
    return nc


def host_prep(x, qkv_w, qn_w, qn_b, kn_w, kn_b, sinks, proj_w, proj_b):
    """Build the 8 per-core input maps (numpy, host-side sharding + tables)."""
    f32 = np.float32
    f16 = np.float16
    n = np.arange(N, dtype=np.float64)
    inv = ROPE_BASE ** (-np.arange(0, D, 2, dtype=np.float64) / D)
    freqs = n[:, None] * inv[None, :]
    emb = np.concatenate([freqs, freqs], axis=1)
    cos, sin = np.cos(emb), np.sin(emb)
    sgn = np.concatenate([-np.ones(D // 2), np.ones(D // 2)])

    def tables(w, scale):
        w = np.asarray(w, np.float64) * scale
        w_rot = np.concatenate([w[D // 2:], w[:D // 2]])
        cw = (cos * w[None, :]).astype(f16)
        sw = (sin * w_rot[None, :] * sgn[None, :]).astype(f16)
        return np.ascontiguousarray(cw), np.ascontiguousarray(sw)

    coswq, sinwq = tables(qn_w, 1.0)
    coswk, sinwk = tables(kn_w, np.sqrt(D))  # sqrt(D) fold for k rstd
    assert np.allclose(qn_b, 0) and np.allclose(kn_b, 0), \
        "nonzero qk-norm bias not implemented"

    r = np.arange(P)[:, None]
    c = np.arange(P)[None, :]
    m_up = np.where(c > r, 0.0, NEG)   # window boundary block (kb = qi-2)
    m_lo = np.where(c <= r, 0.0, NEG)  # causal diag block (kb = qi)
    masks_np = np.ascontiguousarray(np.concatenate(
        [m_up, np.zeros((P, P)), m_lo], axis=1).astype(f16))
    identh_np = np.eye(P).astype(f16)

    def center(wrows):
        # per-head mean over d folded into weights: z' = x @ w'.T centered
        w3 = wrows.reshape(-1, D, DIM)
        return (w3 - w3.mean(axis=1, keepdims=True)).reshape(-1, DIM)

    in_maps = []
    for core in range(8):
        b, g = core // 2, core % 2
        q_rows = center(qkv_w[g * 512:(g + 1) * 512])
        k_rows = center(qkv_w[1024 + g * 512:1024 + (g + 1) * 512])
        v_rows = qkv_w[2048 + g * 512:2048 + (g + 1) * 512]
        in_maps.append({
            "xt": np.ascontiguousarray(x[b].T.astype(ml_dtypes.bfloat16)),
            "wqkt": np.ascontiguousarray(
                np.concatenate([q_rows, k_rows], 0).T.astype(ml_dtypes.bfloat16)),
            "wvt": np.ascontiguousarray(v_rows.T.astype(ml_dtypes.bfloat16)),
            "projt": np.ascontiguousarray(
                proj_w[:, g * 512:(g + 1) * 512].T.astype(f16)),
            "coswq": coswq, "sinwq": sinwq,
            "coswk": coswk, "sinwk": sinwk,
            "esink": np.exp(sinks[g * 8:(g + 1) * 8]).astype(f32).reshape(1, HL),
            "masks": masks_np,
            "identh": identh_np,
        })
    return in_maps


def assemble(results, proj_b):
    out = np.zeros((B, N, DIM), dtype=np.float32)
    for b in range(B):
        out[b] = results[2 * b]["y"] + results[2 * b + 1]["y"] + proj_b[None, :]
    return out


# ---------------------------------------------------------------------------
# Public entry point: kernel(**inputs) -> full output [B, N, DIM]
# ---------------------------------------------------------------------------
from concourse.bass_utils import run_bass_kernel_spmd

_NC_CACHE = {}


def _get_nc():
    if "nc" not in _NC_CACHE:
        _NC_CACHE["nc"] = build_nc(repeat=1)
    return _NC_CACHE["nc"]


def kernel(x, qkv_w, qn_w, qn_b, kn_w, kn_b, sinks, proj_w, proj_b):
    x = np.asarray(x, np.float32)
    qkv_w = np.asarray(qkv_w, np.float32)
    proj_w = np.asarray(proj_w, np.float32)
    in_maps = host_prep(x, qkv_w, np.asarray(qn_w), np.asarray(qn_b),
                        np.asarray(kn_w), np.asarray(kn_b),
                        np.asarray(sinks), proj_w, np.asarray(proj_b))
    nc = _get_nc()
    res = run_bass_kernel_spmd(nc, in_maps, core_ids=list(range(8)))
    return assemble(res.results, np.asarray(proj_b, np.float32))


# revision 22
# speedup vs baseline: 2.8296x; 1.0598x over previous
"""Bass/Tile kernel for sparse sliding-window attention with sinks (v2).

Problem (full): B=4, N=1024, DIM=1024, H=16, D=64, SW=256.
Sharding: 8 cores; core c -> batch b=c//2, head-group g=c%2 (8 heads each).
Host sums the two per-head-group partial projections + proj bias.

v2 changes vs baseline:
  - mean-centering folded into qkv weights on host (LN mean path removed;
    var = sumsq/D on centered z)
  - fp16 for all post-QKV tensors (zraw/qkt/v/p/att/proj weights) -> DVE
    2x/4x modes, PE 1 cyc/row
  - rstd via Quake rsqrt on DVE (no ACT Sqrt -> single act table set);
    q-side rstd folded into the softmax exp scale (free), k-side applied
    once per tile with sqrt(D) folded into the k rope tables
  - sliding-window masks applied via a PE accumulation matmul
    (identity-stationary, host mask tile moving) instead of DVE adds
  - softmax 1/den folded into the p-transpose eviction copy
  - RoPE rotate-half via strided DVE reads (no gpsimd copy)
"""

import sys

sys.path.insert(0, "/opt/trn_rl_repo")

import numpy as np
import ml_dtypes

import concourse.bass as bass
import concourse.mybir as mybir
import concourse.tile as tile
from concourse import bacc

F32 = mybir.dt.float32
F16 = mybir.dt.float16
BF16 = mybir.dt.bfloat16
I32 = mybir.dt.int32

B, N, DIM = 4, 1024, 1024
H, D = 16, 64
SW = 256
ROPE_BASE = 10000.0
LN_EPS = 1e-5
P = 128
NT = N // P      # 8 query/n tiles
CC = DIM // P    # 8 contraction chunks
HL = H // 2      # 8 local heads
NEG = -30000.0   # fp16-safe mask value
MAGIC = 0x5EF759DF  # quake magic adjusted for vh = v/2 input
PNORM = "dve"  # which engine normalizes p: pool | dve | act


def build_nc(repeat=1, use_for_i=False, phases="ABC"):
    nc = bacc.Bacc("TRN2", target_bir_lowering=False, debug=False, num_devices=8)

    xt = nc.declare_dram_parameter("xt", [DIM, N], BF16, isOutput=False)
    wqkt = nc.declare_dram_parameter("wqkt", [DIM, 1024], BF16, isOutput=False)
    wvt = nc.declare_dram_parameter("wvt", [DIM, 512], BF16, isOutput=False)
    projt = nc.declare_dram_parameter("projt", [512, DIM], F16, isOutput=False)
    coswq = nc.declare_dram_parameter("coswq", [N, D], F16, isOutput=False)
    sinwq = nc.declare_dram_parameter("sinwq", [N, D], F16, isOutput=False)
    coswk = nc.declare_dram_parameter("coswk", [N, D], F16, isOutput=False)
    sinwk = nc.declare_dram_parameter("sinwk", [N, D], F16, isOutput=False)
    esink = nc.declare_dram_parameter("esink", [1, HL], F32, isOutput=False)
    masks = nc.declare_dram_parameter("masks", [P, 3 * P], F16, isOutput=False)
    identh = nc.declare_dram_parameter("identh", [P, P], F16, isOutput=False)
    y = nc.declare_dram_parameter("y", [N, DIM], F32, isOutput=True)

    with tile.TileContext(nc) as tc:
        with tc.tile_pool(name="consts", bufs=1) as consts:
            wqk_sb = consts.tile([P, CC, 1024], BF16, tag="wqk")
            wqk_src = wqkt.ap().rearrange("(cc p) f -> p cc f", p=P)
            wv_sb = consts.tile([P, CC, 512], BF16, tag="wv")
            wv_src = wvt.ap().rearrange("(cc p) f -> p cc f", p=P)
            for c in range(CC):
                nc.sync.dma_start(out=wqk_sb[:, c, :], in_=wqk_src[:, c, :])
                nc.sync.dma_start(out=wv_sb[:, c, :], in_=wv_src[:, c, :])
            pj_sb = consts.tile([P, 4, DIM], F16, tag="pj")
            pj_src = projt.ap().rearrange("(ch p) e -> p ch e", p=P)
            for ch in range(4):
                nc.sync.dma_start(out=pj_sb[:, ch, :], in_=pj_src[:, ch, :])
            cos_sin = {}
            for nm, t_dram in (
                ("cq", coswq), ("sq", sinwq), ("ck", coswk), ("sk", sinwk),
            ):
                t_sb = consts.tile([P, NT, D], F16, tag=nm)
                nc.sync.dma_start(
                    out=t_sb, in_=t_dram.ap().rearrange("(t p) d -> p t d", p=P))
                cos_sin[nm] = t_sb
            es_sb = consts.tile([P, HL], F32, tag="es")
            nc.sync.dma_start(out=es_sb, in_=esink.ap().to_broadcast([P, HL]))
            mk_sb = consts.tile([P, 3 * P], F16, tag="mk")
            nc.sync.dma_start(out=mk_sb, in_=masks.ap())
            idh_sb = consts.tile([P, P], F16, tag="idh")
            nc.sync.dma_start(out=idh_sb, in_=identh.ap())
            magic_sb = consts.tile([P, 1], I32, tag="magic")
            nc.vector.memset(magic_sb, MAGIC)

            # persistent intermediates
            qkt_sb = consts.tile([P, 8, N], F16, tag="qkt")   # [d, pair, n]
            v_sb = consts.tile([P, NT, 512], F16, tag="v")    # [n, ktile, hd]
            att_sb = consts.tile([P, 4, NT, P], F16, tag="att")  # [hd, pair, qi, n]
            zraw = consts.tile([P, NT, 1024], F16, tag="zraw")   # centered qk
            sq_g = consts.tile([P, NT, 16], F32, tag="sqg")      # sumsq
            y_g = consts.tile([P, NT, 16], F32, tag="yg")        # rsqrt(sumsq+Deps)

            def body(pools, phases=phases):
                if True:
                    pA, psA, pR, pB = pools
                    psB = psA
                    do_ln = ('L' in phases) or ('A' in phases)

                    def a1_tile(t):
                        nsl = slice(t * P, (t + 1) * P)
                        xt_t = pA.tile([P, CC, P], BF16, tag="xt", name=f"xt{t}")
                        nc.sync.dma_start(
                            out=xt_t,
                            in_=xt[:, nsl].rearrange("(cc p) n -> p cc n", p=P))
                        ps_q = psA.tile([P, 512], F32, tag="psq", name=f"psq{t}")
                        ps_k = psA.tile([P, 512], F32, tag="psk", name=f"psk{t}")
                        ps_v = psA.tile([P, 512], F32, tag="psv", name=f"psv{t}")
                        for c in range(CC):
                            st, sp = (c == 0), (c == CC - 1)
                            nc.tensor.matmul(ps_q, xt_t[:, c, :], wqk_sb[:, c, 0:512],
                                             start=st, stop=sp)
                            nc.tensor.matmul(ps_k, xt_t[:, c, :], wqk_sb[:, c, 512:1024],
                                             start=st, stop=sp)
                            nc.tensor.matmul(ps_v, xt_t[:, c, :], wv_sb[:, c, :],
                                             start=st, stop=sp)
                        nc.scalar.copy(out=v_sb[:, t, :], in_=ps_v)
                        if not do_ln:
                            return
                        # evict centered z to fp16 (ACT), sumsq via DVE on fp16
                        nc.scalar.copy(out=zraw[:, t, 0:512], in_=ps_q)
                        nc.scalar.copy(out=zraw[:, t, 512:1024], in_=ps_k)
                        sq = pR.tile([P, 16, D], F16, tag=f"sq{t % 2}",
                                     name=f"sq{t}")
                        z16 = zraw[:, t, :].rearrange("p (s d) -> p s d", d=D)
                        nc.vector.tensor_mul(out=sq, in0=z16, in1=z16)
                        with nc.allow_low_precision("fp16 sumsq ok for rstd"):
                            nc.vector.tensor_reduce(
                                out=sq_g[:, t, :], in_=sq,
                                axis=mybir.AxisListType.X, op=mybir.AluOpType.add)

                    def stats_batch(g):
                        # y = rsqrt(sumsq + D*eps) for tiles [2g, 2g+2) via
                        # quake rsqrt + 2 newton steps (all DVE, no ACT table)
                        tsl = slice(g * 2, g * 2 + 2)
                        ysl = y_g[:, tsl, :]
                        vh = pR.tile([P, 2, 16], F32, tag=f"vh{g % 2}", name=f"vh{g}")
                        yt = pR.tile([P, 2, 16], F32, tag=f"yt{g % 2}", name=f"yt{g}")
                        sc2 = pR.tile([P, 2, 16], F32, tag=f"sc2{g % 2}", name=f"sc2{g}")
                        nc.vector.tensor_scalar(
                            out=vh, in0=sq_g[:, tsl, :], scalar1=0.5,
                            scalar2=0.5 * D * LN_EPS,
                            op0=mybir.AluOpType.mult, op1=mybir.AluOpType.add)
                        ivh = vh.bitcast(I32)
                        iy = yt.bitcast(I32)
                        nc.vector.tensor_scalar(
                            out=iy, in0=ivh, scalar1=1, scalar2=None,
                            op0=mybir.AluOpType.logical_shift_right)
                        nc.vector.tensor_tensor(
                            out=iy, in0=magic_sb.broadcast_to([P, 2, 16]),
                            in1=iy, op=mybir.AluOpType.subtract)
                        for it in range(2):
                            dst = ysl if it == 1 else yt
                            nc.vector.tensor_mul(out=sc2, in0=yt, in1=yt)
                            nc.vector.tensor_mul(out=sc2, in0=sc2, in1=vh)
                            nc.vector.tensor_scalar(
                                out=sc2, in0=sc2, scalar1=-1.0, scalar2=1.5,
                                op0=mybir.AluOpType.mult, op1=mybir.AluOpType.add)
                            nc.vector.tensor_mul(out=dst, in0=yt, in1=sc2)

                    def a3_tiles(ts):
                        for t in ts:
                            rot = pR.tile([P, 1024], F16, tag=f"rot{t % 2}",
                                          name=f"rotb{t}")
                            tmp = pR.tile([P, 1024], F16, tag=f"tmp{t % 2}",
                                          name=f"tmpb{t}")
                            J = D // 2
                            zv = zraw[:, t, :].rearrange(
                                "p (s h j) -> p s h j", h=2, j=J)
                            rv = rot.rearrange("p (s h j) -> p s h j", h=2, j=J)
                            tv = tmp.rearrange("p (s d) -> p s d", d=D)
                            for half, (cn, sn) in enumerate(
                                    (("cq", "sq"), ("ck", "sk"))):
                                ssl = slice(half * 8, half * 8 + 8)
                                cb = cos_sin[cn][:, t, :].unsqueeze(1)\
                                    .broadcast_to([P, 8, D])
                                sb3 = cos_sin[sn][:, t, :]\
                                    .rearrange("p (h j) -> p h j", j=J)
                                # t = z * cos
                                nc.vector.tensor_mul(
                                    out=tv[:, ssl, :],
                                    in0=zraw[:, t, half * 512:(half + 1) * 512]
                                    .rearrange("p (s d) -> p s d", d=D),
                                    in1=cb)
                                # r[j'] = z[1-j'] * sin[j']  (rotate-half)
                                for jp in range(2):
                                    nc.vector.tensor_mul(
                                        out=rv[:, ssl, jp, :],
                                        in0=zv[:, ssl, 1 - jp, :],
                                        in1=sb3[:, jp, :].unsqueeze(1)
                                        .broadcast_to([P, 8, J]))
                            # u = t + r, k-half scaled by y_k after
                            nc.vector.tensor_add(
                                out=zraw[:, t, :], in0=tmp, in1=rot)
                            # k *= y_k (8/sqrt fold in tables)
                            zk = zraw[:, t, 512:1024].rearrange(
                                "p (s d) -> p s d", d=D)
                            nc.vector.tensor_mul(
                                out=zk, in0=zk,
                                in1=y_g[:, t, 8:16].unsqueeze(2)
                                .broadcast_to([P, 8, D]))
                        for t in ts:
                            nsl = slice(t * P, (t + 1) * P)
                            for hp in range(2):  # q half / k half
                                tp = psA.tile([P, 512], F16, tag="tp",
                                              name=f"tp{t}_{hp}")
                                for pr in range(4):
                                    blk = hp * 4 + pr
                                    nc.tensor.transpose(
                                        tp[:, pr * P:(pr + 1) * P],
                                        zraw[:, t, blk * P:(blk + 1) * P],
                                        idh_sb)
                                # evict 4 transposed blocks to qkt
                                for pr in range(4):
                                    if hp == 0:
                                        nc.scalar.copy(
                                            out=qkt_sb[:, hp * 4 + pr, nsl],
                                            in_=tp[:, pr * P:(pr + 1) * P])
                                    else:
                                        nc.vector.tensor_copy(
                                            out=qkt_sb[:, hp * 4 + pr, nsl],
                                            in_=tp[:, pr * P:(pr + 1) * P])

                    # ladder: grain-2 stats; a3(t-3) emitted before a1(t)
                    for t in range(8):
                        if do_ln and t >= 3:
                            a3_tiles([t - 3])
                        a1_tile(t)
                        if do_ln and t % 2 == 1:
                            stats_batch(t // 2)
                    if do_ln:
                        a3_tiles(range(5, 8))

                    if not do_ln or 'B' not in phases:
                        return
                    # ======== phase B+C, software-pipelined over qi
                    digits = [int(ch) for ch in phases if ch.isdigit()]
                    bmax = digits[0] if digits else 9
                    def stage1(qi):
                        qsl = slice(qi * P, (qi + 1) * P)
                        kb0 = max(qi - 2, 0)
                        nkb = qi - kb0 + 1
                        NK = nkb * P
                        den8 = pB.tile([P, HL], F32, tag="den8")
                        rec8 = pB.tile([P, HL], F32, tag="rec8")
                        p_ts, scs = [], []
                        # scores (PE, fp16)
                        for h in range(HL):
                            pair, poff = h // 2, (h % 2) * 64
                            sc = psB.tile([P, 512], F32,
                                          tag="psq" if h % 2 == 0 else "psk",
                                          name=f"sc{h}")[:, 0:3 * P]
                            nc.tensor.matmul(
                                sc[:, 0:NK],
                                qkt_sb[poff:poff + 64, pair, qsl],
                                qkt_sb[poff:poff + 64, 4 + pair,
                                       kb0 * P:kb0 * P + NK],
                                start=True, stop=True)
                            scs.append(sc)
                        # masks via PE accumulation (identity stationary)
                        for h in range(HL):
                            sc = scs[h]
                            if nkb < 3:
                                nc.tensor.matmul(
                                    sc[:, (nkb - 1) * P:NK], idh_sb,
                                    mk_sb[:, 2 * P:3 * P],
                                    start=False, stop=True,
                                    skip_group_check=True)
                            else:
                                nc.tensor.matmul(
                                    sc[:, 0:P], idh_sb, mk_sb[:, 0:P],
                                    start=False, stop=True,
                                    skip_group_check=True)
                                nc.tensor.matmul(
                                    sc[:, 2 * P:NK], idh_sb,
                                    mk_sb[:, 2 * P:3 * P],
                                    start=False, stop=True,
                                    skip_group_check=True)
                        # exp (ACT): scale = y_q, accum -> den
                        for h in range(HL):
                            p_t = pB.tile([P, 3 * P], F16, tag=f"p{h}", bufs=3,
                                          name=f"pt{h}")
                            nc.scalar.activation(
                                out=p_t[:, 0:NK], in_=scs[h][:, 0:NK],
                                func=mybir.ActivationFunctionType.Exp,
                                scale=y_g[:, qi, h:h + 1],
                                accum_out=den8[:, h:h + 1])
                            p_ts.append(p_t)
                        if bmax < 2:
                            return None
                        nc.vector.tensor_add(out=den8, in0=den8, in1=es_sb)
                        nc.vector.reciprocal(out=rec8, in_=den8)
                        # normalize p (DVE/ACT)
                        for h in range(0 if bmax < 3 else HL):
                            p_t = p_ts[h]
                            if PNORM == "dve":
                                nc.vector.tensor_scalar_mul(
                                    out=p_t[:, 0:NK], in0=p_t[:, 0:NK],
                                    scalar1=rec8[:, h:h + 1])
                            elif PNORM == "pool":
                                nc.gpsimd.tensor_scalar_mul(
                                    out=p_t[:, 0:NK], in0=p_t[:, 0:NK],
                                    scalar1=rec8[:, h:h + 1])
                            else:
                                nc.scalar.activation(
                                    out=p_t[:, 0:NK], in_=p_t[:, 0:NK],
                                    func=mybir.ActivationFunctionType.Identity,
                                    scale=rec8[:, h:h + 1])
                        return p_ts

                    def stage2(qi, p_ts):
                        qsl = slice(qi * P, (qi + 1) * P)
                        kb0 = max(qi - 2, 0)
                        nkb = qi - kb0 + 1
                        ptss = []
                        # transpose p (PE) + evict (DVE/ACT)
                        for h in range(0 if bmax < 3 else HL):
                            p_t = p_ts[h]
                            ptp = psB.tile([P, 512], F16, tag="tp",
                                           name=f"ptp{h}")\
                                .rearrange("p (j q) -> p j q", q=P)
                            for j in range(nkb):
                                nc.tensor.transpose(
                                    ptp[:, j, :], p_t[:, j * P:(j + 1) * P],
                                    idh_sb)
                            pts = pB.tile([P, 3, P], F16, tag=f"pts{h % 2}",
                                          bufs=2, name=f"pts{h}")
                            if h % 2 == 0:
                                nc.vector.tensor_copy(out=pts[:, 0:nkb, :],
                                                      in_=ptp[:, 0:nkb, :])
                            else:
                                nc.scalar.copy(out=pts[:, 0:nkb, :],
                                               in_=ptp[:, 0:nkb, :])
                            ptss.append(pts)
                        # PV (PE) + attn evict (ACT/DVE)
                        at = None
                        for h in range(0 if bmax < 4 else HL):
                            pair, poff = h // 2, (h % 2) * 64
                            if h % 2 == 0:
                                at = psB.tile([P, 512], F32, tag="psv",
                                              name=f"at{h}")[:, 0:P]
                            for j in range(nkb):
                                kb = kb0 + j
                                nc.tensor.matmul(
                                    at[poff:poff + 64, :],
                                    v_sb[:, kb, h * D:(h + 1) * D],
                                    ptss[h][:, j, :],
                                    start=(j == 0), stop=(j == nkb - 1))
                            if h % 2 == 1:
                                if pair % 2 == 0:
                                    nc.scalar.copy(
                                        out=att_sb[:, pair, qi, :], in_=at)
                                else:
                                    nc.vector.tensor_copy(
                                        out=att_sb[:, pair, qi, :], in_=at)
                        # ============ phase C: proj
                        for e in range(2 if 'C' in phases else 0):
                            pj_ps = psB.tile([P, 512], F32, tag="psv",
                                             name=f"pj{e}")
                            for ch in range(4):
                                nc.tensor.matmul(
                                    pj_ps,
                                    att_sb[:, ch, qi, :],
                                    pj_sb[:, ch, e * 512:(e + 1) * 512],
                                    start=(ch == 0), stop=(ch == 3))
                            y_sb = pB.tile([P, 512], F32, tag="ysb")
                            if e == 0:
                                nc.scalar.copy(out=y_sb, in_=pj_ps)
                            else:
                                nc.vector.tensor_copy(out=y_sb, in_=pj_ps)
                            nc.sync.dma_start(
                                out=y[qsl, e * 512:(e + 1) * 512], in_=y_sb)

                    prev = None
                    for qi in range(NT):
                        p_ts = stage1(qi)
                        if prev is not None:
                            stage2(qi - 1, prev)
                        prev = p_ts
                    if prev is not None:
                        stage2(NT - 1, prev)

            with (
                tc.tile_pool(name="pA", bufs=3) as pA,
                tc.tile_pool(name="psA", bufs=2, space="PSUM") as psA,
                tc.tile_pool(name="pR", bufs=1) as pR,
                tc.tile_pool(name="pB", bufs=2) as pB,
            ):
                pools = (pA, psA, pR, pB)
                if use_for_i and repeat > 1:
                    with tc.For_i(0, repeat, 1):
                        body(pools)
                else:
                    for _ in range(repeat):
                        body(pools)

    nc.finalize()
    return nc


def host_prep(x, qkv_w, qn_w, qn_b, kn_w, kn_b, sinks, proj_w, proj_b):
    """Build the 8 per-core input maps (numpy, host-side sharding + tables)."""
    f32 = np.float32
    f16 = np.float16
    n = np.arange(N, dtype=np.float64)
    inv = ROPE_BASE ** (-np.arange(0, D, 2, dtype=np.float64) / D)
    freqs = n[:, None] * inv[None, :]
    emb = np.concatenate([freqs, freqs], axis=1)
    cos, sin = np.cos(emb), np.sin(emb)
    sgn = np.concatenate([-np.ones(D // 2), np.ones(D // 2)])

    def tables(w, scale):
        w = np.asarray(w, np.float64) * scale
        w_rot = np.concatenate([w[D // 2:], w[:D // 2]])
        cw = (cos * w[None, :]).astype(f16)
        sw = (sin * w_rot[None, :] * sgn[None, :]).astype(f16)
        return np.ascontiguousarray(cw), np.ascontiguousarray(sw)

    coswq, sinwq = tables(qn_w, 1.0)
    coswk, sinwk = tables(kn_w, np.sqrt(D))  # sqrt(D) fold for k rstd
    assert np.allclose(qn_b, 0) and np.allclose(kn_b, 0), \
        "nonzero qk-norm bias not implemented"

    r = np.arange(P)[:, None]
    c = np.arange(P)[None, :]
    m_up = np.where(c > r, 0.0, NEG)   # window boundary block (kb = qi-2)
    m_lo = np.where(c <= r, 0.0, NEG)  # causal diag block (kb = qi)
    masks_np = np.ascontiguousarray(np.concatenate(
        [m_up, np.zeros((P, P)), m_lo], axis=1).astype(f16))
    identh_np = np.eye(P).astype(f16)

    def center(wrows):
        # per-head mean over d folded into weights: z' = x @ w'.T centered
        w3 = wrows.reshape(-1, D, DIM)
        return (w3 - w3.mean(axis=1, keepdims=True)).reshape(-1, DIM)

    in_maps = []
    for core in range(8):
        b, g = core // 2, core % 2
        q_rows = center(qkv_w[g * 512:(g + 1) * 512])
        k_rows = center(qkv_w[1024 + g * 512:1024 + (g + 1) * 512])
        v_rows = qkv_w[2048 + g * 512:2048 + (g + 1) * 512]
        in_maps.append({
            "xt": np.ascontiguousarray(x[b].T.astype(ml_dtypes.bfloat16)),
            "wqkt": np.ascontiguousarray(
                np.concatenate([q_rows, k_rows], 0).T.astype(ml_dtypes.bfloat16)),
            "wvt": np.ascontiguousarray(v_rows.T.astype(ml_dtypes.bfloat16)),
            "projt": np.ascontiguousarray(
                proj_w[:, g * 512:(g + 1) * 512].T.astype(f16)),
            "coswq": coswq, "sinwq": sinwq,
            "coswk": coswk, "sinwk": sinwk,
            "esink": np.exp(sinks[g * 8:(g + 1) * 8]).astype(f32).reshape(1, HL),
            "masks": masks_np,
            "identh": identh_np,
        })
    return in_maps


def assemble(results, proj_b):
    out = np.zeros((B, N, DIM), dtype=np.float32)
    for b in range(B):
        out[b] = results[2 * b]["y"] + results[2 * b + 1]["y"] + proj_b[None, :]
    return out


# ---------------------------------------------------------------------------
# Public entry point: kernel(**inputs) -> full output [B, N, DIM]
# ---------------------------------------------------------------------------
from concourse.bass_utils import run_bass_kernel_spmd

_NC_CACHE = {}


def _get_nc():
    if "nc" not in _NC_CACHE:
        _NC_CACHE["nc"] = build_nc(repeat=1)
    return _NC_CACHE["nc"]


def kernel(x, qkv_w, qn_w, qn_b, kn_w, kn_b, sinks, proj_w, proj_b):
    x = np.asarray(x, np.float32)
    qkv_w = np.asarray(qkv_w, np.float32)
    proj_w = np.asarray(proj_w, np.float32)
    in_maps = host_prep(x, qkv_w, np.asarray(qn_w), np.asarray(qn_b),
                        np.asarray(kn_w), np.asarray(kn_b),
                        np.asarray(sinks), proj_w, np.asarray(proj_b))
    nc = _get_nc()
    res = run_bass_kernel_spmd(nc, in_maps, core_ids=list(range(8)))
    return assemble(res.results, np.asarray(proj_b, np.float32))
